# revision 20
# baseline (speedup 1.0000x reference)
"""Trainium2 Bass/Tile kernel for nn_MultiHeadAttention (B=4, S=2048, D=1024,
H=16, Dh=64, fp32), SPMD across 8 NeuronCores.

Sharding: core c -> batch c//2, head-half c%2 (8 heads per core).
Host pre-transposes each batch slice to [D, S] and casts to bf16, so the
device needs no transposes: QK projections produce Q^T/K^T [feat, tok]
directly (weight as stationary), the V projection produces V [tok, feat]
with an appended ones-column, scores come out as scores^T [k, q] (two
heads row-packed on the 128-wide contraction via tile_position), exp runs
on the scalar engine with the 1/sqrt(Dh) scale folded in (scores are
bounded ~±3, so no max-subtraction is needed), and the PV matmul uses
V as the stationary operand, yielding out^T plus the softmax denominator
for free from the ones column.  The host divides by the denominator,
adds the V bias (exact because softmax rows sum to 1), transposes, and
reassembles the full [4, 2048, 1024] fp32 output.

Scheduling: the steady state is scalar-engine(exp)-paced (~1.13us per
[128,1024] exp, 256 of them = ~290us busy), so the kernel keeps ACT fed
from the first microseconds to the last:
 - K/Q inputs are loaded ONCE into resident SBUF tiles (an earlier
   version re-loaded them per head-pair, starving ACT ~30us at pair
   boundaries), in token-major chunks issued right before the compute
   that needs them (the wait-merge-onto-LDWEIGHTS pass coarsens DMA
   waits up to the latest already-issued DMA, so issue order matters).
   V streams through a 2-buffer chunk pool on the gpsimd queue.
 - Each q-tile's PV chains are software-pipelined one q-tile behind:
   their 32 matmuls run two-per-score-block inside the NEXT q-tile's
   score loop, so the PE instruction mix per block (1 score pair + 2 PV
   + woven projection matmuls) matches the ACT-paced rate and the next
   exp never queues behind a PV burst.  The last q-tile self-chains
   with lag 8 so only 8 chain steps trail the final exp.
 - Projection matmuls for later pairs are queued as one-matmul weave
   ops popped between blocks; the 23-deep et ring absorbs the V-
   projection burst.
Measured on trn2 (profiled): ~385us vs ~435us for the previous version
of this kernel under identical measurement; rel err ~2.2e-3.
"""

import numpy as np
import ml_dtypes

import concourse.bacc as bacc
import concourse.tile as tile
from concourse import mybir
from concourse.bass_utils import run_bass_kernel_spmd

F32 = mybir.dt.float32
BF16 = mybir.dt.bfloat16
_BF = ml_dtypes.bfloat16

B, S, D, H, DH = 4, 2048, 1024, 16, 64
HH = 8          # heads per core
NP = HH // 2    # head pairs per core
JW = HH * DH    # 512 projected features per core
N_CORES = 8


def _build_nc(S=S, qt_size=512, sc_bufs=2, pv_bufs=2, exp_bufs=23, v_bufs=2):
    KT8 = D // 128
    NQT = S // qt_size
    NKT = S // 128
    NTT = S // 128
    TC = 512
    NTC = S // TC

    nc = bacc.Bacc("TRN2", target_bir_lowering=False, debug=False,
                   num_devices=N_CORES)

    qT = nc.declare_dram_parameter("qT", [D, S], BF16, isOutput=False)
    kT = nc.declare_dram_parameter("kT", [D, S], BF16, isOutput=False)
    vT = nc.declare_dram_parameter("vT", [D, S], BF16, isOutput=False)
    wq = nc.declare_dram_parameter("wq", [D, JW], BF16, isOutput=False)
    wk = nc.declare_dram_parameter("wk", [D, JW], BF16, isOutput=False)
    wv = nc.declare_dram_parameter("wv", [D, JW], BF16, isOutput=False)
    bq = nc.declare_dram_parameter("bq", [JW], F32, isOutput=False)
    bk = nc.declare_dram_parameter("bk", [JW], F32, isOutput=False)
    numT = nc.declare_dram_parameter("numT", [HH, 65, S], F32, isOutput=True)
    w_dram = {"wq": wq, "wk": wk, "wv": wv}
    in_dram = {"q": qT, "k": kT}

    with tile.TileContext(nc) as tc:
        with (
            tc.tile_pool(name="consts", bufs=1) as consts,
            tc.tile_pool(name="persist", bufs=1) as persist,
            tc.tile_pool(name="vins", bufs=v_bufs) as vins,
            tc.tile_pool(name="exps", bufs=exp_bufs) as exps,
            tc.tile_pool(name="ostage", bufs=3) as ostage,
            tc.tile_pool(name="scps", bufs=sc_bufs, space="PSUM") as scps,
            tc.tile_pool(name="pvps", bufs=pv_bufs, space="PSUM") as pvps,
            tc.tile_pool(name="prps", bufs=2, space="PSUM") as prps,
        ):
            w_sb = {}

            def load_w(name, eng=None):
                eng = eng or nc.sync
                t = consts.tile([128, KT8, JW], BF16, tag=name)
                src_r = w_dram[name].ap().rearrange("(kt p) j -> p kt j", p=128)
                for kt in range(KT8):
                    eng.dma_start(out=t[:, kt, :], in_=src_r[:, kt, :])
                w_sb[name] = t

            def load_bias(name, src):
                t = consts.tile([128, NP], F32, tag=name)
                nc.scalar.dma_start(
                    out=t[:], in_=src.ap().rearrange("(pr j) -> j pr", j=128))
                return t

            QT_sb = persist.tile([128, NP, S], BF16, tag="QT")
            KT_sb = persist.tile([128, NP, S], BF16, tag="KT")
            V_aug = persist.tile([128, NTT, HH, 65], BF16, tag="Vaug")
            # resident K/Q inputs: [D-chunk partitions, kt, token]
            IN_sb = {n: persist.tile([128, KT8, S], BF16, tag=f"in_{n}",
                                     name=f"IN_{n}")
                     for n in ("k", "q")}

            def load_in_chunks(name, s):
                """Load token-chunk s (512 tokens) of all 8 D-chunks.
                k goes through the sync queue, q through the vector queue so
                the two input streams land in parallel."""
                t = IN_sb[name]
                eng = nc.sync if name == "k" else nc.scalar
                for kt in range(KT8):
                    eng.dma_start(
                        out=t[:, kt, s * TC:(s + 1) * TC],
                        in_=in_dram[name].ap()[kt * 128:(kt + 1) * 128,
                                               s * TC:(s + 1) * TC])

            def proj_qk_slot(pair, name, s):
                """One token-chunk (one PSUM bank) per projection pass."""
                wname, bias, dst = {
                    "k": ("wk", bias_k, KT_sb), "q": ("wq", bias_q, QT_sb)}[name]
                ps = prps.tile([128, TC], F32, tag="pr",
                               name=f"ps_{pair}_{name}_{s}")
                tc0 = s * TC
                for kt in range(KT8):
                    nc.tensor.matmul(
                        ps[:], w_sb[wname][:, kt, pair * 128:(pair + 1) * 128],
                        IN_sb[name][:, kt, tc0:tc0 + TC],
                        start=(kt == 0), stop=(kt == KT8 - 1))
                nc.vector.tensor_scalar_add(
                    dst[:, pair, tc0:tc0 + TC], ps[:], bias[:, pair:pair + 1])

            def proj_v_tt(tt, vtile, vs):
                """Project one 128-token tile of V (8 matmuls + copy)."""
                ps = prps.tile([128, JW], F32, tag="pr", name=f"psv_{tt}")
                t0 = tt * 128 - vs * TC
                for kt in range(KT8):
                    nc.tensor.matmul(
                        ps[:],
                        vtile[:, kt, t0:t0 + 128],
                        w_sb["wv"][:, kt, :],
                        start=(kt == 0), stop=(kt == KT8 - 1))
                nc.vector.tensor_copy(
                    V_aug[:, tt, :, 0:64],
                    ps[:].rearrange("p (h d) -> p h d", d=64))

            ets = {}

            def attn_scores(pair, qt, kts):
                """Emit (scores, exp) groups for kts; stash et tiles."""
                q0 = qt * qt_size
                for kt in kts:
                    sc = scps.tile([128, 2, qt_size], F32, tag="sc")
                    for h2 in range(2):
                        nc.tensor.matmul(
                            sc[:, h2, :],
                            KT_sb[h2 * 64:(h2 + 1) * 64, pair,
                                  kt * 128:(kt + 1) * 128],
                            QT_sb[h2 * 64:(h2 + 1) * 64, pair, q0:q0 + qt_size],
                            start=True, stop=True)
                    et = exps.tile([128, 2, qt_size], BF16, tag="exp")
                    nc.scalar.activation(
                        et[:].rearrange("p a b -> p (a b)"),
                        sc[:].rearrange("p a b -> p (a b)"),
                        mybir.ActivationFunctionType.Exp, scale=0.125)
                    ets[(pair, qt, kt)] = et

            def chain_start(pair, qt):
                return {"pair": pair, "qt": qt, "pv": [
                    pvps.tile([65, qt_size], F32, tag="pv",
                              name=f"pv_{pair}_{qt}_{h2}")
                    for h2 in range(2)]}

            def chain_step(ch, kt):
                for h2 in range(2):
                    nc.tensor.matmul(
                        ch["pv"][h2][:],
                        V_aug[:, kt, ch["pair"] * 2 + h2, :],
                        ets[(ch["pair"], ch["qt"], kt)][:, h2, :],
                        start=(kt == 0), stop=(kt == NKT - 1))

            def chain_finish(ch):
                q0 = ch["qt"] * qt_size
                for h2 in range(2):
                    ot = ostage.tile([65, qt_size], F32, tag="ot")
                    nc.vector.tensor_copy(ot[:], ch["pv"][h2][:])
                    nc.sync.dma_start(
                        out=numT.ap()[ch["pair"] * 2 + h2, :,
                                      q0:q0 + qt_size],
                        in_=ot[:])
                for kt in range(NKT):
                    del ets[(ch["pair"], ch["qt"], kt)]

            # ---- weave machinery: a list of pending PE-side closures
            # (one matmul each) sprinkled between attention blocks ----
            weave_q = []
            slot_state = {}

            def make_proj_ops(pair, name, s):
                ops = []
                for kt in range(KT8):
                    def op(p=pair, n=name, ss=s, k=kt):
                        wname, bias, dst = {
                            "k": ("wk", bias_k, KT_sb),
                            "q": ("wq", bias_q, QT_sb)}[n]
                        skey = (p, n, ss)
                        if k == 0:
                            slot_state[skey] = prps.tile(
                                [128, TC], F32, tag="pr",
                                name=f"ps_{p}_{n}_{ss}")
                        ps = slot_state[skey]
                        tc0 = ss * TC
                        nc.tensor.matmul(
                            ps[:], w_sb[wname][:, k, p * 128:(p + 1) * 128],
                            IN_sb[n][:, k, tc0:tc0 + TC],
                            start=(k == 0), stop=(k == KT8 - 1))
                        if k == KT8 - 1:
                            nc.vector.tensor_scalar_add(
                                dst[:, p, tc0:tc0 + TC], ps[:],
                                bias[:, p:p + 1])
                    ops.append(op)
                return ops

            def weave(n):
                for _ in range(n):
                    if weave_q:
                        weave_q.pop(0)()

            def attn_qt_fused(pair, qt, prev_ch, per_block=2):
                """Scores+exp for qt, with the PREVIOUS qt's PV-chain
                matmuls fused two-per-block so the PE mix matches the
                ACT-paced steady state; returns this qt's open chain."""
                for kt in range(NKT):
                    attn_scores(pair, qt, [kt])
                    if prev_ch is not None:
                        chain_step(prev_ch, kt)
                    weave(per_block)
                if prev_ch is not None:
                    chain_finish(prev_ch)
                return chain_start(pair, qt)

            # ================= head =================
            # k-side on the sync queue, q/v-side on the vector queue: the
            # two input streams transfer in parallel, and the first score
            # block only needs wk+k_s0 (sync) and wq+q_s0 (vector).
            load_w("wk")                    # sync
            bias_q = load_bias("bq", bq)
            bias_k = load_bias("bk", bk)
            load_in_chunks("k", 0)          # sync
            load_in_chunks("q", 0)
            load_w("wq", nc.scalar)

            vr = vT.ap().rearrange("(kt p) t -> p kt t", p=128)
            vtiles = []

            def load_v_chunk(vs):
                vt_t = vins.tile([128, KT8, TC], BF16, tag="vin",
                                 name=f"vin_{vs}")
                for kt in range(KT8):
                    nc.gpsimd.dma_start(
                        out=vt_t[:, kt, :],
                        in_=vr[:, kt, vs * TC:(vs + 1) * TC])
                vtiles.append(vt_t)

            load_w("wv", nc.gpsimd)
            load_v_chunk(0)                 # gpsimd
            load_v_chunk(1)

            # Issue each chunk-DMA right before the compute that can run
            # once it lands: the wait-merge-onto-LDWEIGHTS pass coarsens an
            # instruction's DMA waits up to the latest already-issued DMA,
            # so DMAs issued after a consumer can no longer delay it.
            proj_qk_slot(0, "k", 0)
            proj_qk_slot(0, "q", 0)
            load_in_chunks("k", 1)          # sync
            load_in_chunks("q", 1)
            attn_scores(0, 0, range(0, 4))
            proj_qk_slot(0, "k", 1)
            proj_qk_slot(0, "q", 1)
            load_in_chunks("k", 2)
            load_in_chunks("q", 2)
            attn_scores(0, 0, range(4, 8))
            nc.vector.memset(V_aug[:, :, :, 64:65], 1.0)
            proj_qk_slot(0, "k", 2)
            proj_qk_slot(0, "q", 2)
            load_in_chunks("k", 3)
            load_in_chunks("q", 3)
            attn_scores(0, 0, range(8, 12))
            proj_qk_slot(0, "k", 3)
            proj_qk_slot(0, "q", 3)
            attn_scores(0, 0, range(12, 16))

            # proj_v woven with qt0's PV chains and qt1's scores:
            # per token-tile tt: project V[tt], advance qt1 scores one block.
            pv0 = {}
            for h2 in range(2):
                pv0[h2] = pvps.tile([65, qt_size], F32, tag="pv",
                                    name=f"pv00_{h2}")
            for tt in range(NTT):
                if tt == 4:
                    load_v_chunk(2)   # reuses buf0 after tt0-3 matmuls
                if tt == 8:
                    load_v_chunk(3)
                proj_v_tt(tt, vtiles[tt // 4], tt // 4)
                attn_scores(0, 1, [tt])
                for h2 in range(2):
                    nc.tensor.matmul(
                        pv0[h2][:],
                        V_aug[:, tt, h2, :],
                        ets[(0, 0, tt)][:, h2, :],
                        start=(tt == 0), stop=(tt == NTT - 1))
            for h2 in range(2):
                ot = ostage.tile([65, qt_size], F32, tag="ot")
                nc.vector.tensor_copy(ot[:], pv0[h2][:])
                nc.sync.dma_start(out=numT.ap()[h2, :, 0:qt_size], in_=ot[:])
            for kt in range(NKT):
                del ets[(0, 0, kt)]

            # queue up all remaining projection work as per-matmul weave ops
            for pair in range(1, NP):
                for name in ("k", "q"):
                    for s in range(NTC):
                        weave_q.extend(make_proj_ops(pair, name, s))

            # qt1's chains ride along with qt2's scores, and so on:
            # each qt's score loop carries the previous qt's PV chains.
            ch = chain_start(0, 1)
            for qt in range(2, NQT):
                ch = attn_qt_fused(0, qt, ch)
            for pair in range(1, NP):
                for qt in range(NQT):
                    if (pair, qt) == (NP - 1, NQT - 1):
                        break
                    ch = attn_qt_fused(pair, qt, ch)
            # Last qt: compress the predecessor's chains two-per-block into
            # the first half of its score loop, then self-chain with lag 8
            # so only 8 chain steps stay exposed after the final exp.
            lp, lq = NP - 1, NQT - 1
            for kt in range(8):
                attn_scores(lp, lq, [kt])
                chain_step(ch, 2 * kt)
                chain_step(ch, 2 * kt + 1)
                weave(2)
            chain_finish(ch)
            ch = chain_start(lp, lq)
            for kt in range(8, NKT):
                attn_scores(lp, lq, [kt])
                chain_step(ch, kt - 8)
                weave(2)
            for kt in range(8, NKT):
                chain_step(ch, kt)
            chain_finish(ch)
            weave(len(weave_q))

    nc.compile()
    return nc


_NC_CACHE = {}


def _get_nc():
    if "nc" not in _NC_CACHE:
        _NC_CACHE["nc"] = _build_nc()
    return _NC_CACHE["nc"]


def _make_in_maps(key, value, query, Wq, bq, Wk, bk, Wv):
    in_maps = []
    for c in range(N_CORES):
        b, hh = c // 2, c % 2
        js = slice(hh * JW, (hh + 1) * JW)
        in_maps.append({
            "qT": np.ascontiguousarray(query[b].T).astype(_BF),
            "kT": np.ascontiguousarray(key[b].T).astype(_BF),
            "vT": np.ascontiguousarray(value[b].T).astype(_BF),
            "wq": np.ascontiguousarray(Wq[:, js]).astype(_BF),
            "wk": np.ascontiguousarray(Wk[:, js]).astype(_BF),
            "wv": np.ascontiguousarray(Wv[:, js]).astype(_BF),
            "bq": np.ascontiguousarray(bq[js], dtype=np.float32),
            "bk": np.ascontiguousarray(bk[js], dtype=np.float32),
        })
    return in_maps


def _assemble(results, bv):
    out = np.empty((B, S, H * DH), np.float32)
    for c in range(N_CORES):
        b, hh = c // 2, c % 2
        numT = results[c]["numT"]
        blk = numT[:, :DH, :] / numT[:, DH:DH + 1, :]
        out[b, :, hh * JW:(hh + 1) * JW] = (
            blk.reshape(JW, S).T + bv[hh * JW:(hh + 1) * JW])
    return out


def kernel(key, value, query, Wq, bq, Wk, bk, Wv, bv, **_run_kwargs):
    key = np.asarray(key, np.float32)
    value = np.asarray(value, np.float32)
    query = np.asarray(query, np.float32)
    nc = _get_nc()
    in_maps = _make_in_maps(key, value, query,
                            np.asarray(Wq, np.float32), np.asarray(bq, np.float32),
                            np.asarray(Wk, np.float32), np.asarray(bk, np.float32),
                            np.asarray(Wv, np.float32))
    res = run_bass_kernel_spmd(nc, in_maps, list(range(N_CORES)), **_run_kwargs)
    out = _assemble(res.results, np.asarray(bv, np.float32))
    if _run_kwargs:
        kernel.last_result = res
    return out


# revision 21
# speedup vs baseline: 1.1600x; 1.1600x over previous
"""Trainium2 Bass/Tile kernel for nn_MultiHeadAttention (B=4, S=2048, D=1024,
H=16, Dh=64, fp32), SPMD across 8 NeuronCores.

Sharding: core c -> batch c//2, head-half c%2 (8 heads per core).
Host pre-transposes each batch slice to [D, S] and casts to bf16, so the
device needs no transposes: QK projections produce Q^T/K^T [feat, tok]
directly (weight as stationary), the V projection produces V [tok, feat]
with an appended ones-column, scores come out as scores^T [k, q] (two
heads row-packed on the 128-wide contraction via tile_position), exp runs
on the scalar engine with the 1/sqrt(Dh) scale folded in (scores are
bounded ~±3, so no max-subtraction is needed), and the PV matmul uses
V as the stationary operand, yielding out^T plus the softmax denominator
for free from the ones column.  The host divides by the denominator,
adds the V bias (exact because softmax rows sum to 1), transposes, and
reassembles the full [4, 2048, 1024] fp32 output.

Scheduling: the steady state is scalar-engine(exp)-paced (~1.13us per
[128,1024] exp, 256 of them = ~290us busy), so the kernel keeps ACT fed
from the first microseconds to the last:
 - K/Q inputs are loaded ONCE into resident SBUF tiles (an earlier
   version re-loaded them per head-pair, starving ACT ~30us at pair
   boundaries), in token-major chunks issued right before the compute
   that needs them (the wait-merge-onto-LDWEIGHTS pass coarsens DMA
   waits up to the latest already-issued DMA, so issue order matters).
   V streams through a 2-buffer chunk pool on the gpsimd queue.
 - Each q-tile's PV chains are software-pipelined one q-tile behind:
   their 32 matmuls run two-per-score-block inside the NEXT q-tile's
   score loop, so the PE instruction mix per block (1 score pair + 2 PV
   + woven projection matmuls) matches the ACT-paced rate and the next
   exp never queues behind a PV burst.  The last q-tile self-chains
   with lag 8 so only 8 chain steps trail the final exp.
 - Projection matmuls for later pairs are queued as one-matmul weave
   ops popped between blocks; the 23-deep et ring absorbs the V-
   projection burst.
Measured on trn2 (profiled): ~385us vs ~435us for the previous version
of this kernel under identical measurement; rel err ~2.2e-3.
"""

import numpy as np
import ml_dtypes

import concourse.bacc as bacc
import concourse.tile as tile
from concourse import mybir
from concourse.bass_utils import run_bass_kernel_spmd

F32 = mybir.dt.float32
BF16 = mybir.dt.bfloat16
_BF = ml_dtypes.bfloat16

B, S, D, H, DH = 4, 2048, 1024, 16, 64
HH = 8          # heads per core
NP = HH // 2    # head pairs per core
JW = HH * DH    # 512 projected features per core
N_CORES = 8


def _build_nc(S=S, qt_size=512, sc_bufs=2, pv_bufs=2, exp_bufs=23, v_bufs=2):
    KT8 = D // 128
    NQT = S // qt_size
    NKT = S // 128
    NTT = S // 128
    TC = 512
    NTC = S // TC

    nc = bacc.Bacc("TRN2", target_bir_lowering=False, debug=False,
                   num_devices=N_CORES)

    qT = nc.declare_dram_parameter("qT", [D, S], BF16, isOutput=False)
    kT = nc.declare_dram_parameter("kT", [D, S], BF16, isOutput=False)
    vT = nc.declare_dram_parameter("vT", [D, S], BF16, isOutput=False)
    wq = nc.declare_dram_parameter("wq", [D, JW], BF16, isOutput=False)
    wk = nc.declare_dram_parameter("wk", [D, JW], BF16, isOutput=False)
    wv = nc.declare_dram_parameter("wv", [D, JW], BF16, isOutput=False)
    bq = nc.declare_dram_parameter("bq", [JW], F32, isOutput=False)
    bk = nc.declare_dram_parameter("bk", [JW], F32, isOutput=False)
    numT = nc.declare_dram_parameter("numT", [HH, 65, S], F32, isOutput=True)
    w_dram = {"wq": wq, "wk": wk, "wv": wv}
    in_dram = {"q": qT, "k": kT}

    with tile.TileContext(nc) as tc:
        with (
            tc.tile_pool(name="consts", bufs=1) as consts,
            tc.tile_pool(name="persist", bufs=1) as persist,
            tc.tile_pool(name="vins", bufs=v_bufs) as vins,
            tc.tile_pool(name="exps", bufs=exp_bufs) as exps,
            tc.tile_pool(name="ostage", bufs=3) as ostage,
            tc.tile_pool(name="scps", bufs=sc_bufs, space="PSUM") as scps,
            tc.tile_pool(name="pvps", bufs=pv_bufs, space="PSUM") as pvps,
            tc.tile_pool(name="prps", bufs=2, space="PSUM") as prps,
        ):
            w_sb = {}

            def load_w(name, eng=None):
                eng = eng or nc.sync
                t = consts.tile([128, KT8, JW], BF16, tag=name)
                src_r = w_dram[name].ap().rearrange("(kt p) j -> p kt j", p=128)
                for kt in range(KT8):
                    eng.dma_start(out=t[:, kt, :], in_=src_r[:, kt, :])
                w_sb[name] = t

            def load_bias(name, src):
                t = consts.tile([128, NP], F32, tag=name)
                nc.sync.dma_start(
                    out=t[:], in_=src.ap().rearrange("(pr j) -> j pr", j=128))
                return t

            QT_sb = persist.tile([128, NP, S], BF16, tag="QT")
            KT_sb = persist.tile([128, NP, S], BF16, tag="KT")
            V_aug = persist.tile([128, NTT, HH, 65], BF16, tag="Vaug")
            # resident K/Q inputs: [D-chunk partitions, kt, token]
            IN_sb = {n: persist.tile([128, KT8, S], BF16, tag=f"in_{n}",
                                     name=f"IN_{n}")
                     for n in ("k", "q")}

            def load_in_chunks(name, s):
                """Load token-chunk s (512 tokens) of all 8 D-chunks.
                k goes through the sync queue, q through the vector queue so
                the two input streams land in parallel."""
                t = IN_sb[name]
                eng = nc.sync
                for kt in range(KT8):
                    eng.dma_start(
                        out=t[:, kt, s * TC:(s + 1) * TC],
                        in_=in_dram[name].ap()[kt * 128:(kt + 1) * 128,
                                               s * TC:(s + 1) * TC])

            def proj_qk_slot(pair, name, s):
                """One token-chunk (one PSUM bank) per projection pass."""
                wname, bias, dst = {
                    "k": ("wk", bias_k, KT_sb), "q": ("wq", bias_q, QT_sb)}[name]
                ps = prps.tile([128, TC], F32, tag="pr",
                               name=f"ps_{pair}_{name}_{s}")
                tc0 = s * TC
                for kt in range(KT8):
                    nc.tensor.matmul(
                        ps[:], w_sb[wname][:, kt, pair * 128:(pair + 1) * 128],
                        IN_sb[name][:, kt, tc0:tc0 + TC],
                        start=(kt == 0), stop=(kt == KT8 - 1))
                nc.vector.tensor_scalar_add(
                    dst[:, pair, tc0:tc0 + TC], ps[:], bias[:, pair:pair + 1])

            def proj_v_tt(tt, vtile, vs):
                """Project one 128-token tile of V (8 matmuls + copy)."""
                ps = prps.tile([128, JW], F32, tag="pr", name=f"psv_{tt}")
                t0 = tt * 128 - vs * TC
                for kt in range(KT8):
                    nc.tensor.matmul(
                        ps[:],
                        vtile[:, kt, t0:t0 + 128],
                        w_sb["wv"][:, kt, :],
                        start=(kt == 0), stop=(kt == KT8 - 1))
                nc.vector.tensor_copy(
                    V_aug[:, tt, :, 0:64],
                    ps[:].rearrange("p (h d) -> p h d", d=64))

            ets = {}

            def attn_scores(pair, qt, kts):
                """Emit (scores, exp) groups for kts; stash et tiles."""
                q0 = qt * qt_size
                for kt in kts:
                    sc = scps.tile([128, 2, qt_size], F32, tag="sc")
                    for h2 in range(2):
                        nc.tensor.matmul(
                            sc[:, h2, :],
                            KT_sb[h2 * 64:(h2 + 1) * 64, pair,
                                  kt * 128:(kt + 1) * 128],
                            QT_sb[h2 * 64:(h2 + 1) * 64, pair, q0:q0 + qt_size],
                            start=True, stop=True)
                    et = exps.tile([128, 2, qt_size], BF16, tag="exp")
                    nc.scalar.activation(
                        et[:].rearrange("p a b -> p (a b)"),
                        sc[:].rearrange("p a b -> p (a b)"),
                        mybir.ActivationFunctionType.Exp, scale=0.125)
                    ets[(pair, qt, kt)] = et

            def chain_start(pair, qt):
                return {"pair": pair, "qt": qt, "pv": [
                    pvps.tile([65, qt_size], F32, tag="pv",
                              name=f"pv_{pair}_{qt}_{h2}")
                    for h2 in range(2)]}

            def chain_step(ch, kt):
                for h2 in range(2):
                    nc.tensor.matmul(
                        ch["pv"][h2][:],
                        V_aug[:, kt, ch["pair"] * 2 + h2, :],
                        ets[(ch["pair"], ch["qt"], kt)][:, h2, :],
                        start=(kt == 0), stop=(kt == NKT - 1))

            def chain_finish(ch):
                q0 = ch["qt"] * qt_size
                for h2 in range(2):
                    ot = ostage.tile([65, qt_size], F32, tag="ot")
                    nc.vector.tensor_copy(ot[:], ch["pv"][h2][:])
                    nc.sync.dma_start(
                        out=numT.ap()[ch["pair"] * 2 + h2, :,
                                      q0:q0 + qt_size],
                        in_=ot[:])
                for kt in range(NKT):
                    del ets[(ch["pair"], ch["qt"], kt)]

            # ---- weave machinery: a list of pending PE-side closures
            # (one matmul each) sprinkled between attention blocks ----
            weave_q = []
            slot_state = {}

            def make_proj_ops(pair, name, s):
                ops = []
                for kt in range(KT8):
                    def op(p=pair, n=name, ss=s, k=kt):
                        wname, bias, dst = {
                            "k": ("wk", bias_k, KT_sb),
                            "q": ("wq", bias_q, QT_sb)}[n]
                        skey = (p, n, ss)
                        if k == 0:
                            slot_state[skey] = prps.tile(
                                [128, TC], F32, tag="pr",
                                name=f"ps_{p}_{n}_{ss}")
                        ps = slot_state[skey]
                        tc0 = ss * TC
                        nc.tensor.matmul(
                            ps[:], w_sb[wname][:, k, p * 128:(p + 1) * 128],
                            IN_sb[n][:, k, tc0:tc0 + TC],
                            start=(k == 0), stop=(k == KT8 - 1))
                        if k == KT8 - 1:
                            nc.vector.tensor_scalar_add(
                                dst[:, p, tc0:tc0 + TC], ps[:],
                                bias[:, p:p + 1])
                    ops.append(op)
                return ops

            def weave(n):
                for _ in range(n):
                    if weave_q:
                        weave_q.pop(0)()

            def attn_qt_fused(pair, qt, prev_ch, per_block=2):
                """Scores+exp for qt, with the PREVIOUS qt's PV-chain
                matmuls fused two-per-block so the PE mix matches the
                ACT-paced steady state; returns this qt's open chain."""
                for kt in range(NKT):
                    attn_scores(pair, qt, [kt])
                    if prev_ch is not None:
                        chain_step(prev_ch, kt)
                    weave(per_block)
                if prev_ch is not None:
                    chain_finish(prev_ch)
                return chain_start(pair, qt)

            # ================= head =================
            # k-side on the sync queue, q/v-side on the vector queue: the
            # two input streams transfer in parallel, and the first score
            # block only needs wk+k_s0 (sync) and wq+q_s0 (vector).
            load_w("wk")                    # sync
            bias_q = load_bias("bq", bq)
            bias_k = load_bias("bk", bk)
            load_in_chunks("k", 0)          # sync
            load_in_chunks("q", 0)
            load_w("wq", nc.sync)

            vr = vT.ap().rearrange("(kt p) t -> p kt t", p=128)
            vtiles = []

            def load_v_chunk(vs):
                vt_t = vins.tile([128, KT8, TC], BF16, tag="vin",
                                 name=f"vin_{vs}")
                for kt in range(KT8):
                    nc.gpsimd.dma_start(
                        out=vt_t[:, kt, :],
                        in_=vr[:, kt, vs * TC:(vs + 1) * TC])
                vtiles.append(vt_t)

            load_w("wv", nc.gpsimd)
            load_v_chunk(0)                 # gpsimd
            load_v_chunk(1)

            # Issue each chunk-DMA right before the compute that can run
            # once it lands: the wait-merge-onto-LDWEIGHTS pass coarsens an
            # instruction's DMA waits up to the latest already-issued DMA,
            # so DMAs issued after a consumer can no longer delay it.
            proj_qk_slot(0, "k", 0)
            proj_qk_slot(0, "q", 0)
            load_in_chunks("k", 1)          # sync
            load_in_chunks("q", 1)
            attn_scores(0, 0, range(0, 4))
            proj_qk_slot(0, "k", 1)
            proj_qk_slot(0, "q", 1)
            load_in_chunks("k", 2)
            load_in_chunks("q", 2)
            attn_scores(0, 0, range(4, 8))
            nc.vector.memset(V_aug[:, :, :, 64:65], 1.0)
            proj_qk_slot(0, "k", 2)
            proj_qk_slot(0, "q", 2)
            load_in_chunks("k", 3)
            load_in_chunks("q", 3)
            attn_scores(0, 0, range(8, 12))
            proj_qk_slot(0, "k", 3)
            proj_qk_slot(0, "q", 3)
            attn_scores(0, 0, range(12, 16))

            # proj_v woven with qt0's PV chains and qt1's scores:
            # per token-tile tt: project V[tt], advance qt1 scores one block.
            pv0 = {}
            for h2 in range(2):
                pv0[h2] = pvps.tile([65, qt_size], F32, tag="pv",
                                    name=f"pv00_{h2}")
            for tt in range(NTT):
                if tt == 4:
                    load_v_chunk(2)   # reuses buf0 after tt0-3 matmuls
                if tt == 8:
                    load_v_chunk(3)
                proj_v_tt(tt, vtiles[tt // 4], tt // 4)
                attn_scores(0, 1, [tt])
                for h2 in range(2):
                    nc.tensor.matmul(
                        pv0[h2][:],
                        V_aug[:, tt, h2, :],
                        ets[(0, 0, tt)][:, h2, :],
                        start=(tt == 0), stop=(tt == NTT - 1))
            for h2 in range(2):
                ot = ostage.tile([65, qt_size], F32, tag="ot")
                nc.vector.tensor_copy(ot[:], pv0[h2][:])
                nc.sync.dma_start(out=numT.ap()[h2, :, 0:qt_size], in_=ot[:])
            for kt in range(NKT):
                del ets[(0, 0, kt)]

            # queue up all remaining projection work as per-matmul weave ops
            for pair in range(1, NP):
                for name in ("k", "q"):
                    for s in range(NTC):
                        weave_q.extend(make_proj_ops(pair, name, s))

            # qt1's chains ride along with qt2's scores, and so on:
            # each qt's score loop carries the previous qt's PV chains.
            ch = chain_start(0, 1)
            for qt in range(2, NQT):
                ch = attn_qt_fused(0, qt, ch)
            for pair in range(1, NP):
                for qt in range(NQT):
                    if (pair, qt) == (NP - 1, NQT - 1):
                        break
                    ch = attn_qt_fused(pair, qt, ch)
            # Last qt: compress the predecessor's chains two-per-block into
            # the first half of its score loop, then self-chain with lag 8
            # so only 8 chain steps stay exposed after the final exp.
            lp, lq = NP - 1, NQT - 1
            for kt in range(8):
                attn_scores(lp, lq, [kt])
                chain_step(ch, 2 * kt)
                chain_step(ch, 2 * kt + 1)
                weave(2)
            chain_finish(ch)
            ch = chain_start(lp, lq)
            for kt in range(8, NKT):
                attn_scores(lp, lq, [kt])
                chain_step(ch, kt - 8)
                weave(2)
            for kt in range(8, NKT):
                chain_step(ch, kt)
            chain_finish(ch)
            weave(len(weave_q))

    nc.compile()
    return nc


_NC_CACHE = {}


def _get_nc():
    if "nc" not in _NC_CACHE:
        _NC_CACHE["nc"] = _build_nc()
    return _NC_CACHE["nc"]


def _make_in_maps(key, value, query, Wq, bq, Wk, bk, Wv):
    in_maps = []
    for c in range(N_CORES):
        b, hh = c // 2, c % 2
        js = slice(hh * JW, (hh + 1) * JW)
        in_maps.append({
            "qT": np.ascontiguousarray(query[b].T).astype(_BF),
            "kT": np.ascontiguousarray(key[b].T).astype(_BF),
            "vT": np.ascontiguousarray(value[b].T).astype(_BF),
            "wq": np.ascontiguousarray(Wq[:, js]).astype(_BF),
            "wk": np.ascontiguousarray(Wk[:, js]).astype(_BF),
            "wv": np.ascontiguousarray(Wv[:, js]).astype(_BF),
            "bq": np.ascontiguousarray(bq[js], dtype=np.float32),
            "bk": np.ascontiguousarray(bk[js], dtype=np.float32),
        })
    return in_maps


def _assemble(results, bv):
    out = np.empty((B, S, H * DH), np.float32)
    for c in range(N_CORES):
        b, hh = c // 2, c % 2
        numT = results[c]["numT"]
        blk = numT[:, :DH, :] / numT[:, DH:DH + 1, :]
        out[b, :, hh * JW:(hh + 1) * JW] = (
            blk.reshape(JW, S).T + bv[hh * JW:(hh + 1) * JW])
    return out


def kernel(key, value, query, Wq, bq, Wk, bk, Wv, bv, **_run_kwargs):
    key = np.asarray(key, np.float32)
    value = np.asarray(value, np.float32)
    query = np.asarray(query, np.float32)
    nc = _get_nc()
    in_maps = _make_in_maps(key, value, query,
                            np.asarray(Wq, np.float32), np.asarray(bq, np.float32),
                            np.asarray(Wk, np.float32), np.asarray(bk, np.float32),
                            np.asarray(Wv, np.float32))
    res = run_bass_kernel_spmd(nc, in_maps, list(range(N_CORES)), **_run_kwargs)
    out = _assemble(res.results, np.asarray(bv, np.float32))
    if _run_kwargs:
        kernel.last_result = res
    return out


# revision 24
# speedup vs baseline: 1.1623x; 1.0019x over previous
"""Trainium2 Bass/Tile kernel for nn_MultiHeadAttention (B=4, S=2048, D=1024,
H=16, Dh=64, fp32), SPMD across 8 NeuronCores.

Sharding: core c -> batch c//2, head-half c%2 (8 heads per core).
Host pre-transposes each batch slice to [D, S] and casts to bf16, so the
device needs no transposes: QK projections produce Q^T/K^T [feat, tok]
directly (weight as stationary), the V projection produces V [tok, feat]
with an appended ones-column, scores come out as scores^T [k, q] (two
heads row-packed on the 128-wide contraction via tile_position), exp runs
on the scalar engine with the 1/sqrt(Dh) scale folded in (scores are
bounded ~±3, so no max-subtraction is needed), and the PV matmul uses
V as the stationary operand, yielding out^T plus the softmax denominator
for free from the ones column.  The host divides by the denominator,
adds the V bias (exact because softmax rows sum to 1), transposes, and
reassembles the full [4, 2048, 1024] fp32 output.

Scheduling: the steady state is scalar-engine(exp)-paced (~1.13us per
[128,1024] exp, 256 of them = ~290us busy), so the kernel keeps ACT fed
from the first microseconds to the last:
 - K/Q inputs are loaded ONCE into resident SBUF tiles (an earlier
   version re-loaded them per head-pair, starving ACT ~30us at pair
   boundaries), in token-major chunks issued right before the compute
   that needs them (the wait-merge-onto-LDWEIGHTS pass coarsens DMA
   waits up to the latest already-issued DMA, so issue order matters).
   V streams through a 2-buffer chunk pool on the gpsimd queue.
 - Each q-tile's PV chains are software-pipelined one q-tile behind:
   their 32 matmuls run two-per-score-block inside the NEXT q-tile's
   score loop, so the PE instruction mix per block (1 score pair + 2 PV
   + woven projection matmuls) matches the ACT-paced rate and the next
   exp never queues behind a PV burst.  The last q-tile self-chains
   with lag 8 so only 8 chain steps trail the final exp.
 - Projection matmuls for later pairs are queued as one-matmul weave
   ops popped between blocks; the 23-deep et ring absorbs the V-
   projection burst.  Two projection PSUM banks (prps=2) let adjacent
   slots overlap instead of serializing on the bias-add; the fused PV
   chains only ever hold one open accumulator pair, so pvps=2 suffices.
Measured on trn2 (profiled): ~379us vs ~435us for the previous version
of this kernel under identical measurement; rel err ~2.2e-3.
(Note: DMAs on the scalar queue stall ACT badly (-54us) and DMAs on the
vector queue hang the device -- only sync and gpsimd queues are safe.)
"""

import numpy as np
import ml_dtypes

import concourse.bacc as bacc
import concourse.tile as tile
from concourse import mybir
from concourse.bass_utils import run_bass_kernel_spmd

F32 = mybir.dt.float32
BF16 = mybir.dt.bfloat16
_BF = ml_dtypes.bfloat16

B, S, D, H, DH = 4, 2048, 1024, 16, 64
HH = 8          # heads per core
NP = HH // 2    # head pairs per core
JW = HH * DH    # 512 projected features per core
N_CORES = 8


def _build_nc(S=S, qt_size=512, sc_bufs=2, pv_bufs=2, exp_bufs=23, v_bufs=2):
    KT8 = D // 128
    NQT = S // qt_size
    NKT = S // 128
    NTT = S // 128
    TC = 512
    NTC = S // TC

    nc = bacc.Bacc("TRN2", target_bir_lowering=False, debug=False,
                   num_devices=N_CORES)

    qT = nc.declare_dram_parameter("qT", [D, S], BF16, isOutput=False)
    kT = nc.declare_dram_parameter("kT", [D, S], BF16, isOutput=False)
    vT = nc.declare_dram_parameter("vT", [D, S], BF16, isOutput=False)
    wq = nc.declare_dram_parameter("wq", [D, JW], BF16, isOutput=False)
    wk = nc.declare_dram_parameter("wk", [D, JW], BF16, isOutput=False)
    wv = nc.declare_dram_parameter("wv", [D, JW], BF16, isOutput=False)
    bq = nc.declare_dram_parameter("bq", [JW], F32, isOutput=False)
    bk = nc.declare_dram_parameter("bk", [JW], F32, isOutput=False)
    numT = nc.declare_dram_parameter("numT", [HH, 65, S], F32, isOutput=True)
    w_dram = {"wq": wq, "wk": wk, "wv": wv}
    in_dram = {"q": qT, "k": kT}

    with tile.TileContext(nc) as tc:
        with (
            tc.tile_pool(name="consts", bufs=1) as consts,
            tc.tile_pool(name="persist", bufs=1) as persist,
            tc.tile_pool(name="vins", bufs=v_bufs) as vins,
            tc.tile_pool(name="exps", bufs=exp_bufs) as exps,
            tc.tile_pool(name="ostage", bufs=3) as ostage,
            tc.tile_pool(name="scps", bufs=sc_bufs, space="PSUM") as scps,
            tc.tile_pool(name="pvps", bufs=pv_bufs, space="PSUM") as pvps,
            tc.tile_pool(name="prps", bufs=2, space="PSUM") as prps,
        ):
            w_sb = {}

            def load_w(name, eng=None):
                eng = eng or nc.sync
                t = consts.tile([128, KT8, JW], BF16, tag=name)
                src_r = w_dram[name].ap().rearrange("(kt p) j -> p kt j", p=128)
                for kt in range(KT8):
                    eng.dma_start(out=t[:, kt, :], in_=src_r[:, kt, :])
                w_sb[name] = t

            def load_bias(name, src):
                t = consts.tile([128, NP], F32, tag=name)
                nc.sync.dma_start(
                    out=t[:], in_=src.ap().rearrange("(pr j) -> j pr", j=128))
                return t

            QT_sb = persist.tile([128, NP, S], BF16, tag="QT")
            KT_sb = persist.tile([128, NP, S], BF16, tag="KT")
            V_aug = persist.tile([128, NTT, HH, 65], BF16, tag="Vaug")
            # resident K/Q inputs: [D-chunk partitions, kt, token]
            IN_sb = {n: persist.tile([128, KT8, S], BF16, tag=f"in_{n}",
                                     name=f"IN_{n}")
                     for n in ("k", "q")}

            def load_in_chunks(name, s):
                """Load token-chunk s (512 tokens) of all 8 D-chunks.
                k goes through the sync queue, q through the vector queue so
                the two input streams land in parallel."""
                t = IN_sb[name]
                eng = nc.sync
                for kt in range(KT8):
                    eng.dma_start(
                        out=t[:, kt, s * TC:(s + 1) * TC],
                        in_=in_dram[name].ap()[kt * 128:(kt + 1) * 128,
                                               s * TC:(s + 1) * TC])

            def proj_qk_slot(pair, name, s):
                """One token-chunk (one PSUM bank) per projection pass."""
                wname, bias, dst = {
                    "k": ("wk", bias_k, KT_sb), "q": ("wq", bias_q, QT_sb)}[name]
                ps = prps.tile([128, TC], F32, tag="pr",
                               name=f"ps_{pair}_{name}_{s}")
                tc0 = s * TC
                for kt in range(KT8):
                    nc.tensor.matmul(
                        ps[:], w_sb[wname][:, kt, pair * 128:(pair + 1) * 128],
                        IN_sb[name][:, kt, tc0:tc0 + TC],
                        start=(kt == 0), stop=(kt == KT8 - 1))
                nc.vector.tensor_scalar_add(
                    dst[:, pair, tc0:tc0 + TC], ps[:], bias[:, pair:pair + 1])

            def proj_v_tt(tt, vtile, vs):
                """Project one 128-token tile of V (8 matmuls + copy)."""
                ps = prps.tile([128, JW], F32, tag="pr", name=f"psv_{tt}")
                t0 = tt * 128 - vs * TC
                for kt in range(KT8):
                    nc.tensor.matmul(
                        ps[:],
                        vtile[:, kt, t0:t0 + 128],
                        w_sb["wv"][:, kt, :],
                        start=(kt == 0), stop=(kt == KT8 - 1))
                nc.vector.tensor_copy(
                    V_aug[:, tt, :, 0:64],
                    ps[:].rearrange("p (h d) -> p h d", d=64))

            ets = {}

            def attn_scores(pair, qt, kts):
                """Emit (scores, exp) groups for kts; stash et tiles."""
                q0 = qt * qt_size
                for kt in kts:
                    sc = scps.tile([128, 2, qt_size], F32, tag="sc")
                    for h2 in range(2):
                        nc.tensor.matmul(
                            sc[:, h2, :],
                            KT_sb[h2 * 64:(h2 + 1) * 64, pair,
                                  kt * 128:(kt + 1) * 128],
                            QT_sb[h2 * 64:(h2 + 1) * 64, pair, q0:q0 + qt_size],
                            start=True, stop=True)
                    et = exps.tile([128, 2, qt_size], BF16, tag="exp")
                    nc.scalar.activation(
                        et[:].rearrange("p a b -> p (a b)"),
                        sc[:].rearrange("p a b -> p (a b)"),
                        mybir.ActivationFunctionType.Exp, scale=0.125)
                    ets[(pair, qt, kt)] = et

            def chain_start(pair, qt):
                return {"pair": pair, "qt": qt, "pv": [
                    pvps.tile([65, qt_size], F32, tag="pv",
                              name=f"pv_{pair}_{qt}_{h2}")
                    for h2 in range(2)]}

            def chain_step(ch, kt):
                for h2 in range(2):
                    nc.tensor.matmul(
                        ch["pv"][h2][:],
                        V_aug[:, kt, ch["pair"] * 2 + h2, :],
                        ets[(ch["pair"], ch["qt"], kt)][:, h2, :],
                        start=(kt == 0), stop=(kt == NKT - 1))

            def chain_finish(ch):
                q0 = ch["qt"] * qt_size
                for h2 in range(2):
                    ot = ostage.tile([65, qt_size], F32, tag="ot")
                    nc.vector.tensor_copy(ot[:], ch["pv"][h2][:])
                    nc.sync.dma_start(
                        out=numT.ap()[ch["pair"] * 2 + h2, :,
                                      q0:q0 + qt_size],
                        in_=ot[:])
                for kt in range(NKT):
                    del ets[(ch["pair"], ch["qt"], kt)]

            # ---- weave machinery: a list of pending PE-side closures
            # (one matmul each) sprinkled between attention blocks ----
            weave_q = []
            slot_state = {}

            def make_proj_ops(pair, name, s):
                ops = []
                for kt in range(KT8):
                    def op(p=pair, n=name, ss=s, k=kt):
                        wname, bias, dst = {
                            "k": ("wk", bias_k, KT_sb),
                            "q": ("wq", bias_q, QT_sb)}[n]
                        skey = (p, n, ss)
                        if k == 0:
                            slot_state[skey] = prps.tile(
                                [128, TC], F32, tag="pr",
                                name=f"ps_{p}_{n}_{ss}")
                        ps = slot_state[skey]
                        tc0 = ss * TC
                        nc.tensor.matmul(
                            ps[:], w_sb[wname][:, k, p * 128:(p + 1) * 128],
                            IN_sb[n][:, k, tc0:tc0 + TC],
                            start=(k == 0), stop=(k == KT8 - 1))
                        if k == KT8 - 1:
                            nc.vector.tensor_scalar_add(
                                dst[:, p, tc0:tc0 + TC], ps[:],
                                bias[:, p:p + 1])
                    ops.append(op)
                return ops

            def weave(n):
                for _ in range(n):
                    if weave_q:
                        weave_q.pop(0)()

            def attn_qt_fused(pair, qt, prev_ch, per_block=2):
                """Scores+exp for qt, with the PREVIOUS qt's PV-chain
                matmuls fused two-per-block so the PE mix matches the
                ACT-paced steady state; returns this qt's open chain."""
                for kt in range(NKT):
                    attn_scores(pair, qt, [kt])
                    if prev_ch is not None:
                        chain_step(prev_ch, kt)
                    weave(per_block)
                if prev_ch is not None:
                    chain_finish(prev_ch)
                return chain_start(pair, qt)

            # ================= head =================
            # k-side on the sync queue, q/v-side on the vector queue: the
            # two input streams transfer in parallel, and the first score
            # block only needs wk+k_s0 (sync) and wq+q_s0 (vector).
            load_w("wk")                    # sync
            bias_q = load_bias("bq", bq)
            bias_k = load_bias("bk", bk)
            load_in_chunks("k", 0)          # sync
            load_in_chunks("q", 0)
            load_w("wq", nc.sync)

            vr = vT.ap().rearrange("(kt p) t -> p kt t", p=128)
            vtiles = []

            def load_v_chunk(vs):
                vt_t = vins.tile([128, KT8, TC], BF16, tag="vin",
                                 name=f"vin_{vs}")
                for kt in range(KT8):
                    nc.gpsimd.dma_start(
                        out=vt_t[:, kt, :],
                        in_=vr[:, kt, vs * TC:(vs + 1) * TC])
                vtiles.append(vt_t)

            load_w("wv", nc.gpsimd)
            load_v_chunk(0)                 # gpsimd
            load_v_chunk(1)

            # Issue each chunk-DMA right before the compute that can run
            # once it lands: the wait-merge-onto-LDWEIGHTS pass coarsens an
            # instruction's DMA waits up to the latest already-issued DMA,
            # so DMAs issued after a consumer can no longer delay it.
            proj_qk_slot(0, "k", 0)
            proj_qk_slot(0, "q", 0)
            load_in_chunks("k", 1)          # sync
            load_in_chunks("q", 1)
            attn_scores(0, 0, range(0, 4))
            proj_qk_slot(0, "k", 1)
            proj_qk_slot(0, "q", 1)
            load_in_chunks("k", 2)
            load_in_chunks("q", 2)
            attn_scores(0, 0, range(4, 8))
            nc.vector.memset(V_aug[:, :, :, 64:65], 1.0)
            proj_qk_slot(0, "k", 2)
            proj_qk_slot(0, "q", 2)
            load_in_chunks("k", 3)
            load_in_chunks("q", 3)
            attn_scores(0, 0, range(8, 12))
            proj_qk_slot(0, "k", 3)
            proj_qk_slot(0, "q", 3)
            attn_scores(0, 0, range(12, 16))

            # proj_v woven with qt0's PV chains and qt1's scores:
            # per token-tile tt: project V[tt], advance qt1 scores one block.
            pv0 = {}
            for h2 in range(2):
                pv0[h2] = pvps.tile([65, qt_size], F32, tag="pv",
                                    name=f"pv00_{h2}")
            for tt in range(NTT):
                if tt == 4:
                    load_v_chunk(2)   # reuses buf0 after tt0-3 matmuls
                if tt == 8:
                    load_v_chunk(3)
                proj_v_tt(tt, vtiles[tt // 4], tt // 4)
                attn_scores(0, 1, [tt])
                for h2 in range(2):
                    nc.tensor.matmul(
                        pv0[h2][:],
                        V_aug[:, tt, h2, :],
                        ets[(0, 0, tt)][:, h2, :],
                        start=(tt == 0), stop=(tt == NTT - 1))
            for h2 in range(2):
                ot = ostage.tile([65, qt_size], F32, tag="ot")
                nc.vector.tensor_copy(ot[:], pv0[h2][:])
                nc.sync.dma_start(out=numT.ap()[h2, :, 0:qt_size], in_=ot[:])
            for kt in range(NKT):
                del ets[(0, 0, kt)]

            # queue up all remaining projection work as per-matmul weave ops
            for pair in range(1, NP):
                for name in ("k", "q"):
                    for s in range(NTC):
                        weave_q.extend(make_proj_ops(pair, name, s))

            # qt1's chains ride along with qt2's scores, and so on:
            # each qt's score loop carries the previous qt's PV chains.
            ch = chain_start(0, 1)
            for qt in range(2, NQT):
                ch = attn_qt_fused(0, qt, ch)
            for pair in range(1, NP):
                for qt in range(NQT):
                    if (pair, qt) == (NP - 1, NQT - 1):
                        break
                    ch = attn_qt_fused(pair, qt, ch)
            # Last qt: compress the predecessor's chains two-per-block into
            # the first half of its score loop, then self-chain with lag 8
            # so only 8 chain steps stay exposed after the final exp.
            lp, lq = NP - 1, NQT - 1
            for kt in range(8):
                attn_scores(lp, lq, [kt])
                chain_step(ch, 2 * kt)
                chain_step(ch, 2 * kt + 1)
                weave(2)
            chain_finish(ch)
            ch = chain_start(lp, lq)
            for kt in range(8, NKT):
                attn_scores(lp, lq, [kt])
                chain_step(ch, kt - 8)
                weave(2)
            for kt in range(8, NKT):
                chain_step(ch, kt)
            chain_finish(ch)
            weave(len(weave_q))

    nc.compile()
    return nc


_NC_CACHE = {}


def _get_nc():
    if "nc" not in _NC_CACHE:
        _NC_CACHE["nc"] = _build_nc()
    return _NC_CACHE["nc"]


def _make_in_maps(key, value, query, Wq, bq, Wk, bk, Wv):
    in_maps = []
    for c in range(N_CORES):
        b, hh = c // 2, c % 2
        js = slice(hh * JW, (hh + 1) * JW)
        in_maps.append({
            "qT": np.ascontiguousarray(query[b].T).astype(_BF),
            "kT": np.ascontiguousarray(key[b].T).astype(_BF),
            "vT": np.ascontiguousarray(value[b].T).astype(_BF),
            "wq": np.ascontiguousarray(Wq[:, js]).astype(_BF),
            "wk": np.ascontiguousarray(Wk[:, js]).astype(_BF),
            "wv": np.ascontiguousarray(Wv[:, js]).astype(_BF),
            "bq": np.ascontiguousarray(bq[js], dtype=np.float32),
            "bk": np.ascontiguousarray(bk[js], dtype=np.float32),
        })
    return in_maps


def _assemble(results, bv):
    out = np.empty((B, S, H * DH), np.float32)
    for c in range(N_CORES):
        b, hh = c // 2, c % 2
        numT = results[c]["numT"]
        blk = numT[:, :DH, :] / numT[:, DH:DH + 1, :]
        out[b, :, hh * JW:(hh + 1) * JW] = (
            blk.reshape(JW, S).T + bv[hh * JW:(hh + 1) * JW])
    return out


def kernel(key, value, query, Wq, bq, Wk, bk, Wv, bv, **_run_kwargs):
    key = np.asarray(key, np.float32)
    value = np.asarray(value, np.float32)
    query = np.asarray(query, np.float32)
    nc = _get_nc()
    in_maps = _make_in_maps(key, value, query,
                            np.asarray(Wq, np.float32), np.asarray(bq, np.float32),
                            np.asarray(Wk, np.float32), np.asarray(bk, np.float32),
                            np.asarray(Wv, np.float32))
    res = run_bass_kernel_spmd(nc, in_maps, list(range(N_CORES)), **_run_kwargs)
    out = _assemble(res.results, np.asarray(bv, np.float32))
    if _run_kwargs:
        kernel.last_result = res
    return out


# revision 25
# speedup vs baseline: 1.1755x; 1.0114x over previous
"""Trainium2 Bass/Tile kernel for nn_MultiHeadAttention (B=4, S=2048, D=1024,
H=16, Dh=64, fp32), SPMD across 8 NeuronCores.

Sharding: core c -> batch c//2, head-half c%2 (8 heads per core).
Host pre-transposes each batch slice to [D, S] and casts to bf16, so the
device needs no transposes: QK projections produce Q^T/K^T [feat, tok]
directly (weight as stationary), the V projection produces V [tok, feat]
with an appended ones-column, scores come out as scores^T [k, q] (two
heads row-packed on the 128-wide contraction via tile_position), exp runs
on the scalar engine with the 1/sqrt(Dh) scale folded in (scores are
bounded ~±3, so no max-subtraction is needed), and the PV matmul uses
V as the stationary operand, yielding out^T plus the softmax denominator
for free from the ones column.  The host divides by the denominator,
adds the V bias (exact because softmax rows sum to 1), transposes, and
reassembles the full [4, 2048, 1024] fp32 output.

Scheduling: the steady state is scalar-engine(exp)-paced (~1.13us per
[128,1024] exp, 256 of them = ~290us busy), so the kernel keeps ACT fed
from the first microseconds to the last:
 - K/Q inputs are loaded ONCE into resident SBUF tiles (an earlier
   version re-loaded them per head-pair, starving ACT ~30us at pair
   boundaries), in token-major chunks issued right before the compute
   that needs them (the wait-merge-onto-LDWEIGHTS pass coarsens DMA
   waits up to the latest already-issued DMA, so issue order matters).
   V streams through a 2-buffer chunk pool on the gpsimd queue.
 - Each q-tile's PV chains are software-pipelined one q-tile behind:
   their 32 matmuls run two-per-score-block inside the NEXT q-tile's
   score loop, so the PE instruction mix per block (1 score pair + 2 PV
   + woven projection matmuls) matches the ACT-paced rate and the next
   exp never queues behind a PV burst.  The last q-tile self-chains
   with lag 8 so only 8 chain steps trail the final exp.
 - Projection matmuls for later pairs are queued as one-matmul weave
   ops popped between blocks; the 23-deep et ring absorbs the V-
   projection burst.  Two projection PSUM banks (prps=2) let adjacent
   slots overlap instead of serializing on the bias-add; the fused PV
   chains only ever hold one open accumulator pair, so pvps=2 suffices.
Measured on trn2 (profiled): ~379us vs ~435us for the previous version
of this kernel under identical measurement; rel err ~2.2e-3.
(Note: DMAs on the scalar queue stall ACT badly (-54us) and DMAs on the
vector queue hang the device -- only sync and gpsimd queues are safe.)
"""

import numpy as np
import ml_dtypes

import concourse.bacc as bacc
import concourse.tile as tile
from concourse import mybir
from concourse.bass_utils import run_bass_kernel_spmd

F32 = mybir.dt.float32
BF16 = mybir.dt.bfloat16
_BF = ml_dtypes.bfloat16

B, S, D, H, DH = 4, 2048, 1024, 16, 64
HH = 8          # heads per core
NP = HH // 2    # head pairs per core
JW = HH * DH    # 512 projected features per core
N_CORES = 8


def _build_nc(S=S, qt_size=512, sc_bufs=2, pv_bufs=2, exp_bufs=23, v_bufs=2):
    KT8 = D // 128
    NQT = S // qt_size
    NKT = S // 128
    NTT = S // 128
    TC = 512
    NTC = S // TC

    nc = bacc.Bacc("TRN2", target_bir_lowering=False, debug=False,
                   num_devices=N_CORES)

    qT = nc.declare_dram_parameter("qT", [D, S], BF16, isOutput=False)
    kT = nc.declare_dram_parameter("kT", [D, S], BF16, isOutput=False)
    vT = nc.declare_dram_parameter("vT", [D, S], BF16, isOutput=False)
    wq = nc.declare_dram_parameter("wq", [D, JW], BF16, isOutput=False)
    wk = nc.declare_dram_parameter("wk", [D, JW], BF16, isOutput=False)
    wv = nc.declare_dram_parameter("wv", [D, JW], BF16, isOutput=False)
    bq = nc.declare_dram_parameter("bq", [JW], F32, isOutput=False)
    bk = nc.declare_dram_parameter("bk", [JW], F32, isOutput=False)
    numT = nc.declare_dram_parameter("numT", [HH, 65, S], F32, isOutput=True)
    w_dram = {"wq": wq, "wk": wk, "wv": wv}
    in_dram = {"q": qT, "k": kT}

    with tile.TileContext(nc) as tc:
        with (
            tc.tile_pool(name="consts", bufs=1) as consts,
            tc.tile_pool(name="persist", bufs=1) as persist,
            tc.tile_pool(name="vins", bufs=v_bufs) as vins,
            tc.tile_pool(name="exps", bufs=exp_bufs) as exps,
            tc.tile_pool(name="ostage", bufs=3) as ostage,
            tc.tile_pool(name="scps", bufs=sc_bufs, space="PSUM") as scps,
            tc.tile_pool(name="pvps", bufs=pv_bufs, space="PSUM") as pvps,
            tc.tile_pool(name="prps", bufs=2, space="PSUM") as prps,
        ):
            w_sb = {}

            def load_w(name, eng=None):
                eng = eng or nc.sync
                t = consts.tile([128, KT8, JW], BF16, tag=name)
                src_r = w_dram[name].ap().rearrange("(kt p) j -> p kt j", p=128)
                for kt in range(KT8):
                    eng.dma_start(out=t[:, kt, :], in_=src_r[:, kt, :])
                w_sb[name] = t

            def load_bias(name, src):
                t = consts.tile([128, NP], F32, tag=name)
                nc.sync.dma_start(
                    out=t[:], in_=src.ap().rearrange("(pr j) -> j pr", j=128))
                return t

            QT_sb = persist.tile([128, NP, S], BF16, tag="QT")
            KT_sb = persist.tile([128, NP, S], BF16, tag="KT")
            V_aug = persist.tile([128, NTT, HH, 65], BF16, tag="Vaug")
            # resident K/Q inputs: [D-chunk partitions, kt, token]
            IN_sb = {n: persist.tile([128, KT8, S], BF16, tag=f"in_{n}",
                                     name=f"IN_{n}")
                     for n in ("k", "q")}

            def load_in_chunks(name, s):
                """Load token-chunk s (512 tokens) of all 8 D-chunks.
                k goes through the sync queue, q through the vector queue so
                the two input streams land in parallel."""
                t = IN_sb[name]
                eng = nc.sync
                for kt in range(KT8):
                    eng.dma_start(
                        out=t[:, kt, s * TC:(s + 1) * TC],
                        in_=in_dram[name].ap()[kt * 128:(kt + 1) * 128,
                                               s * TC:(s + 1) * TC])

            def proj_qk_slot(pair, name, s):
                """One token-chunk (one PSUM bank) per projection pass."""
                wname, bias, dst = {
                    "k": ("wk", bias_k, KT_sb), "q": ("wq", bias_q, QT_sb)}[name]
                ps = prps.tile([128, TC], F32, tag="pr",
                               name=f"ps_{pair}_{name}_{s}")
                tc0 = s * TC
                for kt in range(KT8):
                    nc.tensor.matmul(
                        ps[:], w_sb[wname][:, kt, pair * 128:(pair + 1) * 128],
                        IN_sb[name][:, kt, tc0:tc0 + TC],
                        start=(kt == 0), stop=(kt == KT8 - 1))
                nc.vector.tensor_scalar_add(
                    dst[:, pair, tc0:tc0 + TC], ps[:], bias[:, pair:pair + 1])

            def proj_v_tt(tt, vtile, vs):
                """Project one 128-token tile of V (8 matmuls + copy)."""
                ps = prps.tile([128, JW], F32, tag="pr", name=f"psv_{tt}")
                t0 = tt * 128 - vs * TC
                for kt in range(KT8):
                    nc.tensor.matmul(
                        ps[:],
                        vtile[:, kt, t0:t0 + 128],
                        w_sb["wv"][:, kt, :],
                        start=(kt == 0), stop=(kt == KT8 - 1))
                nc.vector.tensor_copy(
                    V_aug[:, tt, :, 0:64],
                    ps[:].rearrange("p (h d) -> p h d", d=64))

            ets = {}

            def attn_scores(pair, qt, kts):
                """Emit (scores, exp) groups for kts; stash et tiles."""
                q0 = qt * qt_size
                for kt in kts:
                    sc = scps.tile([128, 2, qt_size], F32, tag="sc")
                    for h2 in range(2):
                        nc.tensor.matmul(
                            sc[:, h2, :],
                            KT_sb[h2 * 64:(h2 + 1) * 64, pair,
                                  kt * 128:(kt + 1) * 128],
                            QT_sb[h2 * 64:(h2 + 1) * 64, pair, q0:q0 + qt_size],
                            start=True, stop=True)
                    et = exps.tile([128, 2, qt_size], BF16, tag="exp")
                    nc.scalar.activation(
                        et[:].rearrange("p a b -> p (a b)"),
                        sc[:].rearrange("p a b -> p (a b)"),
                        mybir.ActivationFunctionType.Exp, scale=0.125)
                    ets[(pair, qt, kt)] = et

            def chain_start(pair, qt):
                return {"pair": pair, "qt": qt, "pv": [
                    pvps.tile([65, qt_size], F32, tag="pv",
                              name=f"pv_{pair}_{qt}_{h2}")
                    for h2 in range(2)]}

            def chain_step(ch, kt):
                for h2 in range(2):
                    nc.tensor.matmul(
                        ch["pv"][h2][:],
                        V_aug[:, kt, ch["pair"] * 2 + h2, :],
                        ets[(ch["pair"], ch["qt"], kt)][:, h2, :],
                        start=(kt == 0), stop=(kt == NKT - 1))

            def chain_finish(ch):
                q0 = ch["qt"] * qt_size
                for h2 in range(2):
                    ot = ostage.tile([65, qt_size], F32, tag="ot")
                    nc.vector.tensor_copy(ot[:], ch["pv"][h2][:])
                    nc.sync.dma_start(
                        out=numT.ap()[ch["pair"] * 2 + h2, :,
                                      q0:q0 + qt_size],
                        in_=ot[:])
                for kt in range(NKT):
                    del ets[(ch["pair"], ch["qt"], kt)]

            # ---- weave machinery: a list of pending PE-side closures
            # (one matmul each) sprinkled between attention blocks ----
            weave_q = []
            slot_state = {}

            def make_proj_ops(pair, name, s):
                ops = []
                for kt in range(KT8):
                    def op(p=pair, n=name, ss=s, k=kt):
                        wname, bias, dst = {
                            "k": ("wk", bias_k, KT_sb),
                            "q": ("wq", bias_q, QT_sb)}[n]
                        skey = (p, n, ss)
                        if k == 0:
                            slot_state[skey] = prps.tile(
                                [128, TC], F32, tag="pr",
                                name=f"ps_{p}_{n}_{ss}")
                        ps = slot_state[skey]
                        tc0 = ss * TC
                        nc.tensor.matmul(
                            ps[:], w_sb[wname][:, k, p * 128:(p + 1) * 128],
                            IN_sb[n][:, k, tc0:tc0 + TC],
                            start=(k == 0), stop=(k == KT8 - 1))
                        if k == KT8 - 1:
                            nc.vector.tensor_scalar_add(
                                dst[:, p, tc0:tc0 + TC], ps[:],
                                bias[:, p:p + 1])
                    ops.append(op)
                return ops

            def weave(n):
                for _ in range(n):
                    if weave_q:
                        weave_q.pop(0)()

            def attn_qt_fused(pair, qt, prev_ch, per_block=2):
                """Scores+exp for qt, with the PREVIOUS qt's PV-chain
                matmuls fused two-per-block so the PE mix matches the
                ACT-paced steady state; returns this qt's open chain."""
                for kt in range(NKT):
                    attn_scores(pair, qt, [kt])
                    if prev_ch is not None:
                        chain_step(prev_ch, kt)
                    weave(per_block)
                if prev_ch is not None:
                    chain_finish(prev_ch)
                return chain_start(pair, qt)

            # ================= head =================
            # k-side on the sync queue, q/v-side on the vector queue: the
            # two input streams transfer in parallel, and the first score
            # block only needs wk+k_s0 (sync) and wq+q_s0 (vector).
            bias_q = load_bias("bq", bq)    # sync (tiny)
            bias_k = load_bias("bk", bk)
            load_w("wk", nc.gpsimd)         # off the critical sync path
            load_in_chunks("q", 0)          # sync
            load_w("wq", nc.sync)
            load_in_chunks("k", 0)          # sync

            vr = vT.ap().rearrange("(kt p) t -> p kt t", p=128)
            vtiles = []

            def load_v_chunk(vs):
                vt_t = vins.tile([128, KT8, TC], BF16, tag="vin",
                                 name=f"vin_{vs}")
                for kt in range(KT8):
                    nc.gpsimd.dma_start(
                        out=vt_t[:, kt, :],
                        in_=vr[:, kt, vs * TC:(vs + 1) * TC])
                vtiles.append(vt_t)

            load_w("wv", nc.gpsimd)
            load_v_chunk(0)                 # gpsimd
            load_v_chunk(1)

            # Issue each chunk-DMA right before the compute that can run
            # once it lands: the wait-merge-onto-LDWEIGHTS pass coarsens an
            # instruction's DMA waits up to the latest already-issued DMA,
            # so DMAs issued after a consumer can no longer delay it.
            proj_qk_slot(0, "q", 0)
            proj_qk_slot(0, "k", 0)
            load_in_chunks("k", 1)          # sync
            load_in_chunks("q", 1)
            attn_scores(0, 0, range(0, 4))
            proj_qk_slot(0, "k", 1)
            proj_qk_slot(0, "q", 1)
            load_in_chunks("k", 2)
            load_in_chunks("q", 2)
            attn_scores(0, 0, range(4, 8))
            nc.vector.memset(V_aug[:, :, :, 64:65], 1.0)
            proj_qk_slot(0, "k", 2)
            proj_qk_slot(0, "q", 2)
            load_in_chunks("k", 3)
            load_in_chunks("q", 3)
            attn_scores(0, 0, range(8, 12))
            proj_qk_slot(0, "k", 3)
            proj_qk_slot(0, "q", 3)
            attn_scores(0, 0, range(12, 16))

            # proj_v woven with qt0's PV chains and qt1's scores:
            # per token-tile tt: project V[tt], advance qt1 scores one block.
            pv0 = {}
            for h2 in range(2):
                pv0[h2] = pvps.tile([65, qt_size], F32, tag="pv",
                                    name=f"pv00_{h2}")
            for tt in range(NTT):
                if tt == 4:
                    load_v_chunk(2)   # reuses buf0 after tt0-3 matmuls
                if tt == 8:
                    load_v_chunk(3)
                proj_v_tt(tt, vtiles[tt // 4], tt // 4)
                attn_scores(0, 1, [tt])
                for h2 in range(2):
                    nc.tensor.matmul(
                        pv0[h2][:],
                        V_aug[:, tt, h2, :],
                        ets[(0, 0, tt)][:, h2, :],
                        start=(tt == 0), stop=(tt == NTT - 1))
            for h2 in range(2):
                ot = ostage.tile([65, qt_size], F32, tag="ot")
                nc.vector.tensor_copy(ot[:], pv0[h2][:])
                nc.sync.dma_start(out=numT.ap()[h2, :, 0:qt_size], in_=ot[:])
            for kt in range(NKT):
                del ets[(0, 0, kt)]

            # queue up all remaining projection work as per-matmul weave ops
            for pair in range(1, NP):
                for name in ("k", "q"):
                    for s in range(NTC):
                        weave_q.extend(make_proj_ops(pair, name, s))

            # qt1's chains ride along with qt2's scores, and so on:
            # each qt's score loop carries the previous qt's PV chains.
            ch = chain_start(0, 1)
            for qt in range(2, NQT):
                ch = attn_qt_fused(0, qt, ch)
            for pair in range(1, NP):
                for qt in range(NQT):
                    if (pair, qt) == (NP - 1, NQT - 1):
                        break
                    ch = attn_qt_fused(pair, qt, ch)
            # Last qt: compress the predecessor's chains two-per-block into
            # the first half of its score loop, then self-chain with lag 8
            # so only 8 chain steps stay exposed after the final exp.
            lp, lq = NP - 1, NQT - 1
            for kt in range(8):
                attn_scores(lp, lq, [kt])
                chain_step(ch, 2 * kt)
                chain_step(ch, 2 * kt + 1)
                weave(2)
            chain_finish(ch)
            ch = chain_start(lp, lq)
            for kt in range(8, NKT):
                attn_scores(lp, lq, [kt])
                chain_step(ch, kt - 8)
                weave(2)
            for kt in range(8, NKT):
                chain_step(ch, kt)
            chain_finish(ch)
            weave(len(weave_q))

    nc.compile()
    return nc


_NC_CACHE = {}


def _get_nc():
    if "nc" not in _NC_CACHE:
        _NC_CACHE["nc"] = _build_nc()
    return _NC_CACHE["nc"]


def _make_in_maps(key, value, query, Wq, bq, Wk, bk, Wv):
    in_maps = []
    for c in range(N_CORES):
        b, hh = c // 2, c % 2
        js = slice(hh * JW, (hh + 1) * JW)
        in_maps.append({
            "qT": np.ascontiguousarray(query[b].T).astype(_BF),
            "kT": np.ascontiguousarray(key[b].T).astype(_BF),
            "vT": np.ascontiguousarray(value[b].T).astype(_BF),
            "wq": np.ascontiguousarray(Wq[:, js]).astype(_BF),
            "wk": np.ascontiguousarray(Wk[:, js]).astype(_BF),
            "wv": np.ascontiguousarray(Wv[:, js]).astype(_BF),
            "bq": np.ascontiguousarray(bq[js], dtype=np.float32),
            "bk": np.ascontiguousarray(bk[js], dtype=np.float32),
        })
    return in_maps


def _assemble(results, bv):
    out = np.empty((B, S, H * DH), np.float32)
    for c in range(N_CORES):
        b, hh = c // 2, c % 2
        numT = results[c]["numT"]
        blk = numT[:, :DH, :] / numT[:, DH:DH + 1, :]
        out[b, :, hh * JW:(hh + 1) * JW] = (
            blk.reshape(JW, S).T + bv[hh * JW:(hh + 1) * JW])
    return out


def kernel(key, value, query, Wq, bq, Wk, bk, Wv, bv, **_run_kwargs):
    key = np.asarray(key, np.float32)
    value = np.asarray(value, np.float32)
    query = np.asarray(query, np.float32)
    nc = _get_nc()
    in_maps = _make_in_maps(key, value, query,
                            np.asarray(Wq, np.float32), np.asarray(bq, np.float32),
                            np.asarray(Wk, np.float32), np.asarray(bk, np.float32),
                            np.asarray(Wv, np.float32))
    res = run_bass_kernel_spmd(nc, in_maps, list(range(N_CORES)), **_run_kwargs)
    out = _assemble(res.results, np.asarray(bv, np.float32))
    if _run_kwargs:
        kernel.last_result = res
    return out


# revision 27
# speedup vs baseline: 1.2080x; 1.0277x over previous
"""Trainium2 Bass/Tile kernel for nn_MultiHeadAttention (B=4, S=2048, D=1024,
H=16, Dh=64, fp32), SPMD across 8 NeuronCores.

Sharding: core c -> batch c//2, head-half c%2 (8 heads per core).
Host pre-transposes each batch slice to [D, S] and casts to bf16, so the
device needs no transposes: QK projections produce Q^T/K^T [feat, tok]
directly (weight as stationary), the V projection produces V [tok, feat]
with an appended ones-column, scores come out as scores^T [k, q] (two
heads row-packed on the 128-wide contraction via tile_position), exp runs
on the scalar engine with the 1/sqrt(Dh) scale folded in (scores are
bounded ~±3, so no max-subtraction is needed), and the PV matmul uses
V as the stationary operand, yielding out^T plus the softmax denominator
for free from the ones column.  The host divides by the denominator,
adds the V bias (exact because softmax rows sum to 1), transposes, and
reassembles the full [4, 2048, 1024] fp32 output.

Scheduling: the steady state is scalar-engine(exp)-paced (~1.13us per
[128,1024] exp, 256 of them = ~290us busy), so the kernel keeps ACT fed
from the first microseconds to the last:
 - K/Q inputs are loaded ONCE into resident SBUF tiles (an earlier
   version re-loaded them per head-pair, starving ACT ~30us at pair
   boundaries), in token-major chunks issued right before the compute
   that needs them (the wait-merge-onto-LDWEIGHTS pass coarsens DMA
   waits up to the latest already-issued DMA, so issue order matters).
   V streams through a 2-buffer chunk pool on the gpsimd queue.
 - Each q-tile's PV chains are software-pipelined one q-tile behind:
   their 32 matmuls run two-per-score-block inside the NEXT q-tile's
   score loop, so the PE instruction mix per block (1 score pair + 2 PV
   + woven projection matmuls) matches the ACT-paced rate and the next
   exp never queues behind a PV burst.  The last q-tile self-chains
   with lag 8 so only 8 chain steps trail the final exp.
 - Projection matmuls for later pairs are queued as one-matmul weave
   ops popped between blocks; the 23-deep et ring absorbs the V-
   projection burst.  Two projection PSUM banks (prps=2) let adjacent
   slots overlap instead of serializing on the bias-add; the fused PV
   chains only ever hold one open accumulator pair, so pvps=2 suffices.
Head: bias/q0/wq/k0 stream the critical sync queue in consumption
order while wk rides the gpsimd queue with the V traffic.
Measured on trn2 (profiled): ~374us vs ~435us for the previous version
of this kernel under identical measurement; rel err ~2.2e-3.
(Note: DMAs on the scalar queue stall ACT badly (-54us) and DMAs on the
vector queue hang the device -- only sync and gpsimd queues are safe.)
"""

import numpy as np
import ml_dtypes

import concourse.bacc as bacc
import concourse.tile as tile
from concourse import mybir
from concourse.bass_utils import run_bass_kernel_spmd

F32 = mybir.dt.float32
BF16 = mybir.dt.bfloat16
_BF = ml_dtypes.bfloat16

B, S, D, H, DH = 4, 2048, 1024, 16, 64
HH = 8          # heads per core
NP = HH // 2    # head pairs per core
JW = HH * DH    # 512 projected features per core
N_CORES = 8


def _build_nc(S=S, qt_size=512, sc_bufs=2, pv_bufs=2, exp_bufs=23, v_bufs=2):
    KT8 = D // 128
    NQT = S // qt_size
    NKT = S // 128
    NTT = S // 128
    TC = 512
    NTC = S // TC

    nc = bacc.Bacc("TRN2", target_bir_lowering=False, debug=False,
                   num_devices=N_CORES)

    qT = nc.declare_dram_parameter("qT", [D, S], BF16, isOutput=False)
    kT = nc.declare_dram_parameter("kT", [D, S], BF16, isOutput=False)
    vT = nc.declare_dram_parameter("vT", [D, S], BF16, isOutput=False)
    wq = nc.declare_dram_parameter("wq", [D, JW], BF16, isOutput=False)
    wk = nc.declare_dram_parameter("wk", [D, JW], BF16, isOutput=False)
    wv = nc.declare_dram_parameter("wv", [D, JW], BF16, isOutput=False)
    bq = nc.declare_dram_parameter("bq", [JW], F32, isOutput=False)
    bk = nc.declare_dram_parameter("bk", [JW], F32, isOutput=False)
    numT = nc.declare_dram_parameter("numT", [HH, 65, S], F32, isOutput=True)
    w_dram = {"wq": wq, "wk": wk, "wv": wv}
    in_dram = {"q": qT, "k": kT}

    with tile.TileContext(nc) as tc:
        with (
            tc.tile_pool(name="consts", bufs=1) as consts,
            tc.tile_pool(name="persist", bufs=1) as persist,
            tc.tile_pool(name="vins", bufs=v_bufs) as vins,
            tc.tile_pool(name="exps", bufs=exp_bufs) as exps,
            tc.tile_pool(name="ostage", bufs=3) as ostage,
            tc.tile_pool(name="scps", bufs=sc_bufs, space="PSUM") as scps,
            tc.tile_pool(name="pvps", bufs=pv_bufs, space="PSUM") as pvps,
            tc.tile_pool(name="prps", bufs=2, space="PSUM") as prps,
        ):
            w_sb = {}

            def load_w(name, eng=None):
                t = consts.tile([128, KT8, JW], BF16, tag=name)
                src_r = w_dram[name].ap().rearrange("(kt p) j -> p kt j", p=128)
                for kt in range(KT8):
                    e = eng or (nc.sync if kt % 2 == 0 else nc.gpsimd)
                    e.dma_start(out=t[:, kt, :], in_=src_r[:, kt, :])
                w_sb[name] = t

            def load_bias(name, src):
                t = consts.tile([128, NP], F32, tag=name)
                nc.sync.dma_start(
                    out=t[:], in_=src.ap().rearrange("(pr j) -> j pr", j=128))
                return t

            QT_sb = persist.tile([128, NP, S], BF16, tag="QT")
            KT_sb = persist.tile([128, NP, S], BF16, tag="KT")
            V_aug = persist.tile([128, NTT, HH, 65], BF16, tag="Vaug")
            # resident K/Q inputs: [D-chunk partitions, kt, token]
            IN_sb = {n: persist.tile([128, KT8, S], BF16, tag=f"in_{n}",
                                     name=f"IN_{n}")
                     for n in ("k", "q")}

            def load_in_chunks(name, s):
                """Load token-chunk s (512 tokens) of all 8 D-chunks.
                k goes through the sync queue, q through the vector queue so
                the two input streams land in parallel."""
                t = IN_sb[name]
                for kt in range(KT8):
                    eng = nc.sync if kt % 2 == 0 else nc.gpsimd
                    eng.dma_start(
                        out=t[:, kt, s * TC:(s + 1) * TC],
                        in_=in_dram[name].ap()[kt * 128:(kt + 1) * 128,
                                               s * TC:(s + 1) * TC])

            def proj_qk_slot(pair, name, s):
                """One token-chunk (one PSUM bank) per projection pass."""
                wname, bias, dst = {
                    "k": ("wk", bias_k, KT_sb), "q": ("wq", bias_q, QT_sb)}[name]
                ps = prps.tile([128, TC], F32, tag="pr",
                               name=f"ps_{pair}_{name}_{s}")
                tc0 = s * TC
                for kt in range(KT8):
                    nc.tensor.matmul(
                        ps[:], w_sb[wname][:, kt, pair * 128:(pair + 1) * 128],
                        IN_sb[name][:, kt, tc0:tc0 + TC],
                        start=(kt == 0), stop=(kt == KT8 - 1))
                nc.vector.tensor_scalar_add(
                    dst[:, pair, tc0:tc0 + TC], ps[:], bias[:, pair:pair + 1])

            def proj_v_tt(tt, vtile, vs):
                """Project one 128-token tile of V (8 matmuls + copy)."""
                ps = prps.tile([128, JW], F32, tag="pr", name=f"psv_{tt}")
                t0 = tt * 128 - vs * TC
                for kt in range(KT8):
                    nc.tensor.matmul(
                        ps[:],
                        vtile[:, kt, t0:t0 + 128],
                        w_sb["wv"][:, kt, :],
                        start=(kt == 0), stop=(kt == KT8 - 1))
                nc.vector.tensor_copy(
                    V_aug[:, tt, :, 0:64],
                    ps[:].rearrange("p (h d) -> p h d", d=64))

            ets = {}

            def attn_scores(pair, qt, kts):
                """Emit (scores, exp) groups for kts; stash et tiles."""
                q0 = qt * qt_size
                for kt in kts:
                    sc = scps.tile([128, 2, qt_size], F32, tag="sc")
                    for h2 in range(2):
                        nc.tensor.matmul(
                            sc[:, h2, :],
                            KT_sb[h2 * 64:(h2 + 1) * 64, pair,
                                  kt * 128:(kt + 1) * 128],
                            QT_sb[h2 * 64:(h2 + 1) * 64, pair, q0:q0 + qt_size],
                            start=True, stop=True)
                    et = exps.tile([128, 2, qt_size], BF16, tag="exp")
                    nc.scalar.activation(
                        et[:].rearrange("p a b -> p (a b)"),
                        sc[:].rearrange("p a b -> p (a b)"),
                        mybir.ActivationFunctionType.Exp, scale=0.125)
                    ets[(pair, qt, kt)] = et

            def chain_start(pair, qt):
                return {"pair": pair, "qt": qt, "pv": [
                    pvps.tile([65, qt_size], F32, tag="pv",
                              name=f"pv_{pair}_{qt}_{h2}")
                    for h2 in range(2)]}

            def chain_step(ch, kt):
                for h2 in range(2):
                    nc.tensor.matmul(
                        ch["pv"][h2][:],
                        V_aug[:, kt, ch["pair"] * 2 + h2, :],
                        ets[(ch["pair"], ch["qt"], kt)][:, h2, :],
                        start=(kt == 0), stop=(kt == NKT - 1))

            def chain_finish(ch):
                q0 = ch["qt"] * qt_size
                for h2 in range(2):
                    ot = ostage.tile([65, qt_size], F32, tag="ot")
                    nc.vector.tensor_copy(ot[:], ch["pv"][h2][:])
                    nc.sync.dma_start(
                        out=numT.ap()[ch["pair"] * 2 + h2, :,
                                      q0:q0 + qt_size],
                        in_=ot[:])
                for kt in range(NKT):
                    del ets[(ch["pair"], ch["qt"], kt)]

            # ---- weave machinery: a list of pending PE-side closures
            # (one matmul each) sprinkled between attention blocks ----
            weave_q = []
            slot_state = {}

            def make_proj_ops(pair, name, s):
                ops = []
                for kt in range(KT8):
                    def op(p=pair, n=name, ss=s, k=kt):
                        wname, bias, dst = {
                            "k": ("wk", bias_k, KT_sb),
                            "q": ("wq", bias_q, QT_sb)}[n]
                        skey = (p, n, ss)
                        if k == 0:
                            slot_state[skey] = prps.tile(
                                [128, TC], F32, tag="pr",
                                name=f"ps_{p}_{n}_{ss}")
                        ps = slot_state[skey]
                        tc0 = ss * TC
                        nc.tensor.matmul(
                            ps[:], w_sb[wname][:, k, p * 128:(p + 1) * 128],
                            IN_sb[n][:, k, tc0:tc0 + TC],
                            start=(k == 0), stop=(k == KT8 - 1))
                        if k == KT8 - 1:
                            nc.vector.tensor_scalar_add(
                                dst[:, p, tc0:tc0 + TC], ps[:],
                                bias[:, p:p + 1])
                    ops.append(op)
                return ops

            def weave(n):
                for _ in range(n):
                    if weave_q:
                        weave_q.pop(0)()

            def attn_qt_fused(pair, qt, prev_ch, per_block=2):
                """Scores+exp for qt, with the PREVIOUS qt's PV-chain
                matmuls fused two-per-block so the PE mix matches the
                ACT-paced steady state; returns this qt's open chain."""
                for kt in range(NKT):
                    attn_scores(pair, qt, [kt])
                    if prev_ch is not None:
                        chain_step(prev_ch, kt)
                    weave(per_block)
                if prev_ch is not None:
                    chain_finish(prev_ch)
                return chain_start(pair, qt)

            # ================= head =================
            # k-side on the sync queue, q/v-side on the vector queue: the
            # two input streams transfer in parallel, and the first score
            # block only needs wk+k_s0 (sync) and wq+q_s0 (vector).
            bias_q = load_bias("bq", bq)    # sync (tiny)
            bias_k = load_bias("bk", bk)
            load_in_chunks("q", 0)          # all loads split sync/gpsimd
            load_w("wq")
            load_w("wk")
            load_in_chunks("k", 0)

            vr = vT.ap().rearrange("(kt p) t -> p kt t", p=128)
            vtiles = []

            def load_v_chunk(vs):
                vt_t = vins.tile([128, KT8, TC], BF16, tag="vin",
                                 name=f"vin_{vs}")
                for kt in range(KT8):
                    eng = nc.sync if kt % 2 == 0 else nc.gpsimd
                    eng.dma_start(
                        out=vt_t[:, kt, :],
                        in_=vr[:, kt, vs * TC:(vs + 1) * TC])
                vtiles.append(vt_t)

            # wv/v0/v1 are issued later, at their need-time positions.

            # Issue each chunk-DMA right before the compute that can run
            # once it lands: the wait-merge-onto-LDWEIGHTS pass coarsens an
            # instruction's DMA waits up to the latest already-issued DMA,
            # so DMAs issued after a consumer can no longer delay it.
            proj_qk_slot(0, "q", 0)
            proj_qk_slot(0, "k", 0)
            load_in_chunks("k", 1)
            load_in_chunks("q", 1)
            attn_scores(0, 0, range(0, 4))
            proj_qk_slot(0, "k", 1)
            proj_qk_slot(0, "q", 1)
            load_w("wv")
            load_in_chunks("k", 2)
            load_in_chunks("q", 2)
            attn_scores(0, 0, range(4, 8))
            nc.vector.memset(V_aug[:, :, :, 64:65], 1.0)
            proj_qk_slot(0, "k", 2)
            proj_qk_slot(0, "q", 2)
            load_v_chunk(0)
            load_in_chunks("k", 3)
            load_in_chunks("q", 3)
            attn_scores(0, 0, range(8, 12))
            proj_qk_slot(0, "k", 3)
            proj_qk_slot(0, "q", 3)
            load_v_chunk(1)
            attn_scores(0, 0, range(12, 16))

            # proj_v woven with qt0's PV chains and qt1's scores:
            # per token-tile tt: project V[tt], advance qt1 scores one block.
            pv0 = {}
            for h2 in range(2):
                pv0[h2] = pvps.tile([65, qt_size], F32, tag="pv",
                                    name=f"pv00_{h2}")
            for tt in range(NTT):
                if tt == 4:
                    load_v_chunk(2)   # reuses buf0 after tt0-3 matmuls
                if tt == 8:
                    load_v_chunk(3)
                proj_v_tt(tt, vtiles[tt // 4], tt // 4)
                attn_scores(0, 1, [tt])
                for h2 in range(2):
                    nc.tensor.matmul(
                        pv0[h2][:],
                        V_aug[:, tt, h2, :],
                        ets[(0, 0, tt)][:, h2, :],
                        start=(tt == 0), stop=(tt == NTT - 1))
            for h2 in range(2):
                ot = ostage.tile([65, qt_size], F32, tag="ot")
                nc.vector.tensor_copy(ot[:], pv0[h2][:])
                nc.sync.dma_start(out=numT.ap()[h2, :, 0:qt_size], in_=ot[:])
            for kt in range(NKT):
                del ets[(0, 0, kt)]

            # queue up all remaining projection work as per-matmul weave ops
            for pair in range(1, NP):
                for name in ("k", "q"):
                    for s in range(NTC):
                        weave_q.extend(make_proj_ops(pair, name, s))

            # qt1's chains ride along with qt2's scores, and so on:
            # each qt's score loop carries the previous qt's PV chains.
            ch = chain_start(0, 1)
            for qt in range(2, NQT):
                ch = attn_qt_fused(0, qt, ch)
            for pair in range(1, NP):
                for qt in range(NQT):
                    if (pair, qt) == (NP - 1, NQT - 1):
                        break
                    ch = attn_qt_fused(pair, qt, ch)
            # Last qt: compress the predecessor's chains two-per-block into
            # the first half of its score loop, then self-chain with lag 8
            # so only 8 chain steps stay exposed after the final exp.
            lp, lq = NP - 1, NQT - 1
            for kt in range(8):
                attn_scores(lp, lq, [kt])
                chain_step(ch, 2 * kt)
                chain_step(ch, 2 * kt + 1)
                weave(2)
            chain_finish(ch)
            ch = chain_start(lp, lq)
            for kt in range(8, NKT):
                attn_scores(lp, lq, [kt])
                chain_step(ch, kt - 8)
                weave(2)
            for kt in range(8, NKT):
                chain_step(ch, kt)
            chain_finish(ch)
            weave(len(weave_q))

    nc.compile()
    return nc


_NC_CACHE = {}


def _get_nc():
    if "nc" not in _NC_CACHE:
        _NC_CACHE["nc"] = _build_nc()
    return _NC_CACHE["nc"]


def _make_in_maps(key, value, query, Wq, bq, Wk, bk, Wv):
    in_maps = []
    for c in range(N_CORES):
        b, hh = c // 2, c % 2
        js = slice(hh * JW, (hh + 1) * JW)
        in_maps.append({
            "qT": np.ascontiguousarray(query[b].T).astype(_BF),
            "kT": np.ascontiguousarray(key[b].T).astype(_BF),
            "vT": np.ascontiguousarray(value[b].T).astype(_BF),
            "wq": np.ascontiguousarray(Wq[:, js]).astype(_BF),
            "wk": np.ascontiguousarray(Wk[:, js]).astype(_BF),
            "wv": np.ascontiguousarray(Wv[:, js]).astype(_BF),
            "bq": np.ascontiguousarray(bq[js], dtype=np.float32),
            "bk": np.ascontiguousarray(bk[js], dtype=np.float32),
        })
    return in_maps


def _assemble(results, bv):
    out = np.empty((B, S, H * DH), np.float32)
    for c in range(N_CORES):
        b, hh = c // 2, c % 2
        numT = results[c]["numT"]
        blk = numT[:, :DH, :] / numT[:, DH:DH + 1, :]
        out[b, :, hh * JW:(hh + 1) * JW] = (
            blk.reshape(JW, S).T + bv[hh * JW:(hh + 1) * JW])
    return out


def kernel(key, value, query, Wq, bq, Wk, bk, Wv, bv, **_run_kwargs):
    key = np.asarray(key, np.float32)
    value = np.asarray(value, np.float32)
    query = np.asarray(query, np.float32)
    nc = _get_nc()
    in_maps = _make_in_maps(key, value, query,
                            np.asarray(Wq, np.float32), np.asarray(bq, np.float32),
                            np.asarray(Wk, np.float32), np.asarray(bk, np.float32),
                            np.asarray(Wv, np.float32))
    res = run_bass_kernel_spmd(nc, in_maps, list(range(N_CORES)), **_run_kwargs)
    out = _assemble(res.results, np.asarray(bv, np.float32))
    if _run_kwargs:
        kernel.last_result = res
    return out
